# revision 35
# baseline (speedup 1.0000x reference)
"""Causal self-attention with ALiBi for TRN2, 8 NeuronCores.

Sharding: core c -> batch b = c % 4, head-shard hs = c // 4.
Head-shard hs owns global heads {2j + hs : j in 0..7} (interleaved so both
shards see the same mix of ALiBi slopes -> balanced banded-attention work).

All matmul operands are bf16 (fp32 PSUM accumulate): fp32 mode on the PE
disables FWL fast weight loads and halves throughput; bf16 keeps every
LDWEIGHTS on the 4-xbus path and every stream at 1 col/cycle.

Per-core computation (B=1 batch, 8 heads):
  phase 1: QKV projection.  x^T arrives via a handful of contiguous DMAs
    (host pre-interleaves to partition-major [p, (s-chunk, d, s)] layout),
    paced in consumption order.  Q^T/K^T produced in [col, s] layout (head
    pairs packed 64+64 into 128-partition tiles, Q pre-scaled by 1/sqrt(HD)
    via host-scaled Wq); PSUM->SBUF copies ride the ACT engine (idle during
    phase 1) as Identity-with-bias.  V produced in [s, col] layout, head
    stride 128 (col 64 = ones -> the PV matmul yields the unnormalized
    output AND the softmax denominator; the 128-wide stationary slice keeps
    FWL on).
  phase 2: per (head, q-chunk 512): scores S^T[k,q] = K^T.T @ Q^T on PE
    (K=64 contraction, head pairs at base partitions 0/64 -> the pair's
    ldweights+matmuls run concurrently in disjoint row groups), exp on ACT
    with per-partition bias slope*(k - qmid) (the -slope*q part of ALiBi
    cancels in softmax; qmid recentering prevents overflow; CUT=18 band
    truncation skips k-tiles with negligible mass), tril mask multiply on
    DVE, PV accumulation into PSUM over the k band, software-pipelined
    (PIPE=2) so the PE never waits on the exp chain.
  phase 3: normalize (fast approx reciprocal of the ones row + gpsimd
    partition broadcast), out-projection O^T.T @ Wo accumulated over
    feature tiles, stripes alternating between two PSUM pools so the
    PSUM->SBUF->DRAM drain overlaps the next stripe.

Emission interleaves phases: attention for q-chunk qc is emitted right
behind the phase-1 pieces it needs (Q/K s-half qc//2, V tiles <= 4qc+3),
so phase-1 PE work fills attention pipeline bubbles across the first three
q-chunks and the exp table is pre-warmed during the initial DMA window.

Host side: shard/repack inputs, run SPMD on 8 cores, sum the two
head-shards' partial outputs per batch, add bo.
"""

import math

import numpy as np

B, S, D, H = 4, 2048, 1024, 16
HD = D // H
NSLOT = 8          # local heads per core
NQC = 4            # q chunks of 512
NKT = 16           # k tiles of 128
SC = 512
KT = 128
NCORES = 8

# ALiBi slopes for global heads
SLOPES = [2.0 ** (-0.5 * (h + 1)) for h in range(H)]

# band cutoff: terms with slope*(q-k) > CUT carry < e^(score_spread - CUT)
# relative softmax mass; at 18 the dropped tail is ~1e-4 of the denominator,
# below the bf16 matmul noise (measured: CUT 58 -> 25 left rel err unchanged)
CUT = 18.0


def _bt(h):
    """Band width in 128-k-tiles for global head h (delta_max + 1)."""
    d_max = int(math.ceil(CUT / SLOPES[h]))
    return min(NKT, (127 + d_max) // 128 + 1)


def _w(h):
    """Max exp-op width (q columns) for global head h: slope*(W/2) <= 64."""
    s = SLOPES[h]
    if s * 256.0 <= 64.0:
        return 512
    if s * 128.0 <= 64.0:
        return 256
    return 128


# per-slot params = union over the two head shards (program is SPMD-shared)
SLOT_BT = [max(_bt(2 * j), _bt(2 * j + 1)) for j in range(NSLOT)]
SLOT_W = [min(_w(2 * j), _w(2 * j + 1)) for j in range(NSLOT)]


def plan_attention():
    """Enumerate all attention tile ops. Returns (ops, bias_cols) where ops is
    a list of dicts and bias_cols maps (slot, mkey) -> expb column index."""
    bias_cols = {}
    ops = []
    for qc in range(NQC):
        for p in range(4):
            for kt in range(4 * qc + 4):
                for half in (0, 1):
                    j = 2 * p + half
                    bt, w = SLOT_BT[j], SLOT_W[j]
                    lo = max(0, 4 * qc - bt + 1)
                    if kt < lo:
                        continue
                    qs_start = max(4 * qc, kt)
                    qs_end = min(4 * qc + 3, kt + bt - 1)
                    if qs_start > qs_end:
                        continue
                    c0 = 128 * (qs_start - 4 * qc)
                    c1 = 128 * (qs_end - 4 * qc) + 128
                    # exp ops aligned to an absolute w-grid within the qc
                    # chunk: qmid (the recentering constant) must depend only
                    # on the column block, never on kt, so that every term
                    # entering a given column's softmax sum carries the same
                    # exp(-slope*qmid) factor.
                    exps = []
                    for g in range((c0 // w) * w, c1, w):
                        a, e = max(c0, g), min(c1, g + w)
                        if a >= e:
                            continue
                        mkey = (512 * qc + g + w // 2) - 128 * kt
                        col = bias_cols.setdefault((j, mkey), len(bias_cols))
                        exps.append((a, e - a, col))
                    ops.append(dict(qc=qc, p=p, half=half, j=j, kt=kt,
                                    c0=c0, c1=c1, exps=exps,
                                    tril=(kt >= 4 * qc),
                                    first=(kt == lo), last=(kt == 4 * qc + 3)))
    return ops, bias_cols


ATT_OPS, BIAS_COLS = plan_attention()
NBIAS = len(BIAS_COLS)

_nc_cache = None


def build_program():
    global _nc_cache
    if _nc_cache is not None:
        return _nc_cache

    import concourse.bacc as bacc
    import concourse.tile as tile
    from concourse import mybir

    F32 = mybir.dt.float32
    BF16 = mybir.dt.bfloat16
    EXP = mybir.ActivationFunctionType.Exp
    IDENT = mybir.ActivationFunctionType.Identity

    nc = bacc.Bacc("TRN2", target_bir_lowering=False, debug=False,
                   num_devices=NCORES)

    xT_d = nc.dram_tensor("xT", [128, 4 * 8 * SC], BF16,
                          kind="ExternalInput")
    wv_d = nc.dram_tensor("wv", [128, 8 * 512], BF16, kind="ExternalInput")
    wqk_d = nc.dram_tensor("wqk", [128, 8 * 1024], BF16,
                           kind="ExternalInput")
    wo_d = nc.dram_tensor("wo", [512, D], BF16, kind="ExternalInput")
    qkb_d = nc.dram_tensor("qkb", [128, 8], F32, kind="ExternalInput")
    bvr_d = nc.dram_tensor("bvr", [128, 512], F32, kind="ExternalInput")
    expb_d = nc.dram_tensor("expb", [128, max(NBIAS, 1)], F32,
                            kind="ExternalInput")
    tril_d = nc.dram_tensor("tril", [128, 128], BF16, kind="ExternalInput")
    ones_d = nc.dram_tensor("ones64", [1, 64], F32, kind="ExternalInput")
    vones_d = nc.dram_tensor("vones", [128, 8], BF16, kind="ExternalInput")
    out_d = nc.dram_tensor("out_p", [S, D], F32, kind="ExternalOutput")

    with tile.TileContext(nc) as tc:
        with nc.allow_low_precision(reason="bf16 attention kernel"), \
             tc.tile_pool(name="persist", bufs=1) as pp, \
             tc.tile_pool(name="expsp", bufs=8) as expsp, \
             tc.tile_pool(name="rcp", bufs=2) as rcp, \
             tc.tile_pool(name="rbp", bufs=2) as rbp, \
             tc.tile_pool(name="outp", bufs=4) as outp, \
             tc.tile_pool(name="qkps", bufs=2, space="PSUM") as qkps, \
             tc.tile_pool(name="sps", bufs=3, space="PSUM") as sps, \
             tc.tile_pool(name="ops_", bufs=3, space="PSUM") as ops_:

            # ---- persistent tiles ----
            qkT_Q = [pp.tile([128, S], BF16, name=f"qkTQ{p}") for p in range(4)]
            qkT_K = [pp.tile([128, S], BF16, name=f"qkTK{p}") for p in range(4)]
            Vbuf = [pp.tile([128, NSLOT * 128], BF16, name=f"vb{t}")
                    for t in range(NKT)]
            OT = [pp.tile([128, S], BF16, name=f"OT{p}") for p in range(4)]
            wo_t = [pp.tile([128, D], BF16, name=f"wo{d}") for d in range(4)]
            xTb = pp.tile([128, 8 * S], BF16, name="xTb")

            def xsl(d, sc):
                return xTb[:, (sc * 8 + d) * SC:(sc * 8 + d) * SC + SC]
            wvb = pp.tile([128, 8 * 512], BF16, name="wvb")
            wv = [wvb[:, 512 * d:512 * (d + 1)] for d in range(8)]
            wqkb = pp.tile([128, 8 * 1024], BF16, name="wqkb")
            wqk = [wqkb[:, 1024 * d:1024 * (d + 1)] for d in range(8)]
            qkb_t = pp.tile([128, 8], F32, name="qkb_t")
            bvr_t = pp.tile([128, 512], F32, name="bvr_t")
            expb_t = pp.tile([128, max(NBIAS, 1)], F32, name="expb_t")
            tril_t = pp.tile([128, 128], BF16, name="tril_t")
            vones_t = pp.tile([128, 8], BF16, name="vones_t")
            wrm = pp.tile([128, 1], F32, name="wrm")

            nc.gpsimd.dma_start(out=qkb_t, in_=qkb_d[:, :])
            nc.gpsimd.dma_start(out=bvr_t, in_=bvr_d[:, :])
            nc.gpsimd.dma_start(out=expb_t, in_=expb_d[:, :])
            nc.gpsimd.dma_start(out=tril_t, in_=tril_d[:, :])
            nc.gpsimd.dma_start(out=vones_t, in_=vones_d[:, :])

            # warm the exp table set (~2.7us ACT_TABLE_LOAD) during the
            # phase-1 DMA window instead of on the first real exp
            nc.scalar.activation(wrm, qkb_t[:, 0:1], EXP, bias=0.0, scale=0.0)

            # ones columns of Vbuf (col 64 of each 65-wide head group)
            for t in range(NKT):
                ones_view = Vbuf[t].rearrange("p (h c) -> p h c", c=128)[:, :, 64:65]
                nc.vector.tensor_copy(ones_view, vones_t.unsqueeze(2))

            # input DMA: contiguous blocks split in halves so the queue
            # FIFO paces delivery in consumption order (everything issued at
            # once would round-robin across the SDMA engines and delay the
            # first-needed tiles behind the whole 8 MB)
            def dma2(q, dst, src_ap):
                n = dst.shape[-1]
                q.dma_start(out=dst[:, 0:n // 2], in_=src_ap[:, 0:n // 2])
                q.dma_start(out=dst[:, n // 2:n], in_=src_ap[:, n // 2:n])

            dma2(nc.sync, wvb, wv_d)
            dma2(nc.gpsimd, xTb[:, 0:8 * SC], xT_d[:, 0:8 * SC])
            dma2(nc.sync, wqkb, wqk_d)
            dma2(nc.gpsimd, xTb[:, 8 * SC:16 * SC], xT_d[:, 8 * SC:16 * SC])
            dma2(nc.sync, xTb[:, 16 * SC:24 * SC], xT_d[:, 16 * SC:24 * SC])
            dma2(nc.gpsimd, xTb[:, 24 * SC:32 * SC], xT_d[:, 24 * SC:32 * SC])
            for d in range(4):
                nc.sync.dma_start(
                    out=wo_t[d], in_=wo_d[128 * d:128 * (d + 1), :])

            # ---- phase-1 pieces ----
            def v_tile(st):
                psv = qkps.tile([128, 512], F32, name="psv", tag="ps1")
                sc, r = st // 4, 128 * (st % 4)
                for d in range(8):
                    nc.tensor.matmul(
                        psv, xsl(d, sc)[:, r:r + 128], wv[d],
                        start=(d == 0), stop=(d == 7))
                vdst = Vbuf[st].rearrange("p (h c) -> p h c", c=128)[:, :, 0:64]
                nc.vector.tensor_tensor(
                    vdst, psv.rearrange("p (g c) -> p g c", c=64),
                    bvr_t.rearrange("p (g c) -> p g c", c=64),
                    op=mybir.AluOpType.add)

            def emit_m(m, sh):
                # half a QK m-tile: 2 s-chunks in 2 PSUM banks; psum -> qkT
                # copies on ACT (mostly idle during phase 1)
                dst = qkT_Q[m] if m < 4 else qkT_K[m - 4]
                psq = [qkps.tile([128, SC], F32, name="psq", tag="ps1")
                       for _ in range(2)]
                for d in range(8):
                    for si in range(2):
                        nc.tensor.matmul(
                            psq[si], wqk[d][:, 128 * m:128 * (m + 1)],
                            xsl(d, 2 * sh + si),
                            start=(d == 0), stop=(d == 7))
                for si in range(2):
                    s = 2 * sh + si
                    nc.scalar.activation(
                        dst[:, SC * s:SC * (s + 1)], psq[si], IDENT,
                        bias=qkb_t[:, m:m + 1], scale=1.0)

            # ---- attention pieces ----
            op_idx = [0]

            def attn(qc, p):
                psumO = {}
                for half in (0, 1):
                    psumO[half] = ops_.tile([128, SC], F32, name="psumO")
                # group plan ops by kt (both halves adjacent)
                groups = []
                while (op_idx[0] < len(ATT_OPS)
                       and ATT_OPS[op_idx[0]]["qc"] == qc
                       and ATT_OPS[op_idx[0]]["p"] == p):
                    o = ATT_OPS[op_idx[0]]
                    op_idx[0] += 1
                    if groups and groups[-1][0]["kt"] == o["kt"]:
                        groups[-1].append(o)
                    else:
                        groups.append([o])

                # software pipeline: PV(kt) is emitted after scores(kt+PIPE)
                # so the PE never stalls on the exp chain.
                PIPE = 2
                pend = []

                def emit_scores(grp):
                    out = []
                    for o in grp:
                        half, kt = o["half"], o["kt"]
                        c0, c1 = o["c0"], o["c1"]
                        rb0 = 64 * half
                        psS = sps.tile([128, SC], F32, name="psS")
                        nc.tensor.matmul(
                            psS[:, c0:c1],
                            qkT_K[p][rb0:rb0 + 64, 128 * kt:128 * (kt + 1)],
                            qkT_Q[p][rb0:rb0 + 64,
                                     SC * qc + c0:SC * qc + c1],
                            start=True, stop=True)
                        eS = expsp.tile([128, SC], BF16, name="eS")
                        for (a, ww, col) in o["exps"]:
                            nc.scalar.activation(
                                eS[:, a:a + ww], psS[:, a:a + ww], EXP,
                                bias=expb_t[:, col:col + 1], scale=1.0)
                        if o["tril"]:
                            nc.vector.tensor_mul(
                                eS[:, c0:c0 + 128], eS[:, c0:c0 + 128], tril_t)
                        out.append((o, eS))
                    return out

                def emit_pv(ready):
                    for (o, eS) in ready:
                        c0, c1 = o["c0"], o["c1"]
                        nc.tensor.matmul(
                            psumO[o["half"]][0:128, c0:c1],
                            Vbuf[o["kt"]][:, 128 * o["j"]:128 * o["j"] + 128],
                            eS[:, c0:c1],
                            start=o["first"], stop=o["last"])

                for gi, grp in enumerate(groups):
                    pend.append(emit_scores(grp))
                    if len(pend) > PIPE:
                        emit_pv(pend.pop(0))
                for ready in pend:
                    emit_pv(ready)

                # normalize both halves (no PE involvement: fast approx
                # reciprocal + gpsimd partition broadcast)
                for half in (0, 1):
                    ssum = rcp.tile([1, SC], F32, name="ssum")
                    nc.vector.tensor_copy(ssum, psumO[half][64:65, :])
                    rc = rcp.tile([1, SC], F32, name="rc")
                    nc.vector.reciprocal_approx_fast(rc, ssum)
                    rb = rbp.tile([64, SC], F32, name="rb")
                    nc.gpsimd.partition_broadcast(rb, rc)
                    nc.vector.tensor_mul(
                        OT[p][64 * half:64 * half + 64,
                              SC * qc:SC * (qc + 1)],
                        psumO[half][0:64, :],
                        rb)

            def outproj(qc):
                for st in range(4 * qc, 4 * qc + 4):
                    if st % 2 == 0:
                        pse = [qkps.tile([128, SC], F32, name="pse", tag="ps1")
                               for _ in range(2)]
                    else:
                        pse = [sps.tile([128, SC], F32, name="psS")
                               for _ in range(2)]
                    for d in range(4):
                        for e in range(2):
                            nc.tensor.matmul(
                                pse[e],
                                OT[d][:, 128 * st:128 * (st + 1)],
                                wo_t[d][:, SC * e:SC * (e + 1)],
                                start=(d == 0), stop=(d == 3))
                    for e in range(2):
                        ob = outp.tile([128, SC], F32, name="ob")
                        if e == 0:
                            nc.vector.tensor_copy(ob, pse[e])
                        else:
                            nc.scalar.activation(ob, pse[e],
                                                 mybir.ActivationFunctionType.Copy)
                        nc.gpsimd.dma_start(
                            out=out_d[128 * st:128 * (st + 1),
                                      SC * e:SC * (e + 1)],
                            in_=ob)

            # ---- interleaved emission: attention for qc follows exactly the
            # phase-1 pieces it needs (Q/K s-half sh=qc//2, V tiles <= 4qc+3),
            # so phase-1 PE work fills the attention pipeline bubbles across
            # the first three q-chunks.
            MP = [(0, 4), (1, 5), (2, 6), (3, 7)]
            for st in range(4):
                v_tile(st)
            for p in range(4):
                emit_m(MP[p][0], 0)
                emit_m(MP[p][1], 0)
                attn(0, p)
                v_tile(4 + p)
            outproj(0)
            for p in range(4):
                attn(1, p)
                v_tile(8 + p)
                emit_m(MP[p][0], 1)
                emit_m(MP[p][1], 1)
            outproj(1)
            for p in range(4):
                attn(2, p)
                v_tile(12 + p)
            outproj(2)
            for p in range(4):
                attn(3, p)
            outproj(3)

    nc.compile()
    _nc_cache = nc
    return nc


def make_inputs(x, mask, Wqkv, bqkv, Wo, bo):
    """Build the 8 per-core input maps."""
    import ml_dtypes

    BF = ml_dtypes.bfloat16
    x = np.ascontiguousarray(x, dtype=np.float32)
    Wqkv = np.asarray(Wqkv, dtype=np.float32)
    bqkv = np.asarray(bqkv, dtype=np.float32)
    Wo = np.asarray(Wo, dtype=np.float32)

    # diagonal-block mask in [k_partition, q_column] layout: keep k <= q,
    # i.e. partition p <= column c -> UPPER-triangular
    tril = np.triu(np.ones((128, 128), dtype=np.float32)).astype(BF)
    ones64 = np.ones((1, 64), dtype=np.float32)
    vones = np.ones((128, 8), dtype=BF)
    p_idx = np.arange(128, dtype=np.float32)[:, None]

    in_maps = []
    for c in range(NCORES):
        b, hs = c % 4, c // 4
        heads = [2 * j + hs for j in range(NSLOT)]
        # column order: Q cols (slot-major), K cols, V cols
        qcols = np.concatenate(
            [np.arange(0 * D + h * HD, 0 * D + h * HD + HD) for h in heads])
        kcols = np.concatenate(
            [np.arange(1 * D + h * HD, 1 * D + h * HD + HD) for h in heads])
        vcols = np.concatenate(
            [np.arange(2 * D + h * HD, 2 * D + h * HD + HD) for h in heads])
        cols = np.concatenate([qcols, kcols, vcols])
        wqkv = Wqkv[:, cols].copy()
        bq = bqkv[cols].copy()
        wqkv[:, :512] *= 0.125  # fold 1/sqrt(HD) into Q
        bq[:512] *= 0.125

        qkb = bq[:1024].reshape(8, 128).T.copy()       # [128, m-tile]
        bvr = np.broadcast_to(bq[1024:], (128, 512)).copy()

        expb = np.zeros((128, max(NBIAS, 1)), dtype=np.float32)
        for (j, mkey), col in BIAS_COLS.items():
            expb[:, col:col + 1] = SLOPES[2 * j + hs] * (p_idx - mkey)

        rows = np.concatenate(
            [np.arange(h * HD, h * HD + HD) for h in heads])
        wo = Wo[rows, :].copy()

        # pre-interleaved partition-major layouts (one contiguous DMA each):
        # xT: [p, (sc, d, s)], wv/wqk: [p, (d, c)]
        xt = np.ascontiguousarray(x[b].T)                  # [D, S]
        xt = xt.reshape(8, 128, 4, 512).transpose(1, 2, 0, 3).reshape(128, -1)
        wv_il = wqkv[:, 1024:1536].reshape(8, 128, 512)
        wv_il = wv_il.transpose(1, 0, 2).reshape(128, -1)
        wqk_il = wqkv[:, 0:1024].reshape(8, 128, 1024)
        wqk_il = wqk_il.transpose(1, 0, 2).reshape(128, -1)
        in_maps.append({
            "xT": np.ascontiguousarray(xt).astype(BF),
            "wv": np.ascontiguousarray(wv_il).astype(BF),
            "wqk": np.ascontiguousarray(wqk_il).astype(BF),
            "wo": np.ascontiguousarray(wo).astype(BF),
            "qkb": np.ascontiguousarray(qkb),
            "bvr": bvr,
            "expb": expb,
            "tril": tril,
            "ones64": ones64,
            "vones": vones,
        })
    return in_maps


def kernel(x, mask, Wqkv, bqkv, Wo, bo, _trace=False):
    from concourse.bass_utils import run_bass_kernel_spmd

    nc = build_program()
    in_maps = make_inputs(x, mask, Wqkv, bqkv, Wo, bo)
    res = run_bass_kernel_spmd(nc, in_maps, core_ids=list(range(NCORES)),
                               trace=_trace, trace_cores=[0] if _trace else None)
    bo = np.asarray(bo, dtype=np.float32)
    out = np.empty((B, S, D), dtype=np.float32)
    for b in range(B):
        out[b] = res.results[b]["out_p"] + res.results[b + 4]["out_p"] + bo
    if _trace:
        kernel._last_result = res
    return out

